# revision 48
# baseline (speedup 1.0000x reference)
"""Euclidean distance matrix (torch.cdist p=2) on 8 Trainium2 NeuronCores.

Strategy — fp8 residual output, no on-device sqrt (~55us vs 80us for the
fp16+device-sqrt baseline):
  - d^2 = ||a||^2 + ||b||^2 - 2 a.b. Only the cross term needs the device;
    sq1 (per row) and sq2 (per column) are tiny host-side vectors added
    exactly during decode. The device outputs v = (-2 a.b)/S in fp8e4m3
    and the host computes d = sqrt(S*v + sq1_i + sq2_j). cross is
    zero-centered, so fp8 quantization of v adds ~7e-3 rel err on top of
    the ~6e-3 from fp8 matmul inputs (measured 1.35e-2 total vs the 2e-2
    gate; S=48 keeps |v| <= ~4.2, in the fine fp8 octaves).
  - Why: the baseline was walled by Scalar-engine Sqrt (60us of ACTIVATE,
    the only sqrt engine) and by HBM traffic (16.8 MB fp16 out + 2.3 MB
    in ~ 53us floor at ~358 GB/s/core). Dropping sqrt lets BOTH
    PSUM-capable element engines split the PSUM->SBUF drain: ACT runs
    Identity activation with scale=1/S (1.00us per [128,1024] tile,
    (172+FD)/1.2ns) and DVE runs tensor_scalar_mul (1.13us,
    (120+FD)/0.96ns) -- 34.5us each in parallel, the kernel's wall.
    fp8 output halves DMA bytes (8.4 MB out).
  - PE: cross matmuls only (fp8e4m3 DoubleRow, K=2x128, N=512, 2 per
    psum tile), ~35us busy -- slack vs the element wall, which also
    absorbs its HAM clock oscillation (short idle gaps keep it at
    K=4/8 half-clock part of the time).
  - PSUM: each element engine owns a private 2-buf pool of [128,1024]
    (2 banks x 4 = all 8 banks). A shared round-robin pool stalls PE on
    the specific buffer held by the slower engine (~+0.5us/pair); with
    private pools each engine's next buffer refills during its current
    op, and both streams run back-to-back (ACT 95%+, DVE 98%+ occupancy).
  - Loop is h-major (64 tiles, c = h*MB + m): full-rate compute needs
    only a3 (0.25 MB) + the first column chunk, so the input ramp is
    ~6us instead of ~12 (m-major needs all 2.25 MB before m=0 ends).
    Weights then change every tile; the PE's LDWEIGHTS pull-ahead hides
    the reloads (~0.14us effective). Tiles alternate ACT/DVE by parity
    (flips at {9,33} give the 34/30 split matching the 1.00:1.13 rates;
    keeping c=1 on DVE lets both pipelines start immediately).
  - Output: per-m staging tiles [128, 8192] fp8 live all run; 0.25 MB
    pieces per (m, h-pair) leave on gpsimd/sync through the run. h=6,7
    run per-m as pairs with their piece right behind, the last two split
    sync||scalar. gpsimd (SWDGE) carries nothing after h=5: the kernel
    end is gated by its last DMA + ~4us ring drain + ~4us barrier
    cascade. Inputs stream on sync+scalar HWDGE in consumption order
    (512-col first pieces); early SWDGE input traffic would steal SDMA
    round-robin bandwidth from the latency-critical first pieces.
"""

import numpy as np

N1 = 8192  # x1 rows (output rows)
N2 = 8192  # x2 rows (output cols)
D = 256    # feature dim
NCORES = 8
M1 = N1 // NCORES  # 1024 output rows per core
P = 128            # partitions
KS = 2             # fp8 DoubleRow k-subtiles (K = KS*P = 256)
NT = 512           # matmul moving free dim (one PSUM bank)
PW = 1024          # psum tile width (2 banks); 4 bufs = full PSUM
MB = M1 // P       # 8 output-row blocks per core
HB = 8             # column tiles per row block (1024 cols each)
HW = N2 // HB      # 1024
S = 48.0           # fp8 output scale: v = psum/S, |v| <= ~4.2

# per-tile element-engine assignment (c = h*MB + m, 64 tiles):
# DVE on odd tiles except {9,33} -> 34 ACT / 30 DVE, matching the
# measured 1.00 (ACT) : 1.13 (DVE) us/tile rates. 61<->62 swapped so
# the slower DVE stream gets work later into the tail (tile 63 is
# split across both engines).
DVE_TILES = frozenset(
    (c for c in range(64) if c % 2 == 1 and c not in (9, 33, 61)),
) | {62}

_built = None
_decode = None  # (sq1, sq2) stashed by _prep_inputs for _postprocess


def _ldw_key(inst):
    return (
        str(inst.ins[0]),
        str(getattr(inst, "perf_mode", None)),
        str(getattr(inst, "tile_position", None)),
    )


def _dedupe_ldweights(nc):
    """Drop InstLdweights whose weights AP equals the currently-loaded one
    (no different load in between on the PE stream). Their rare sync waits
    are migrated to the next PE instruction; Bacc.finalize() later splits
    any resulting multi-wait into EventSemaphore preludes."""
    import concourse.mybir as mybir

    dropped = 0
    for f in nc.m.functions:
        for blk in f.blocks:
            insts = list(blk.instructions)
            cur_key = None
            pending = []
            to_drop = []
            for inst in insts:
                if isinstance(inst, mybir.InstLdweights):
                    key = _ldw_key(inst)
                    if key == cur_key:
                        si = inst.sync_info
                        if si is not None and si.on_wait:
                            pending.extend(si.on_wait)
                        to_drop.append(inst)
                    else:
                        cur_key = key
                elif isinstance(inst, mybir.InstMatmult):
                    if pending:
                        si = inst.sync_info
                        waits = list(si.on_wait) if si else []
                        upds = list(si.on_update) if si else []
                        inst.sync_info = mybir.SyncInfo(
                            on_wait=waits + pending, on_update=upds
                        )
                        pending = []
            assert not pending
            for inst in to_drop:
                blk.instructions.remove(inst)
            dropped += len(to_drop)
    return dropped


def _build_nc():
    import concourse.bass as bass
    import concourse.mybir as mybir
    from concourse import bacc, tile

    f8 = mybir.dt.float8e4
    f32 = mybir.dt.float32
    DR = mybir.MatmulPerfMode.DoubleRow
    Ident = mybir.ActivationFunctionType.Identity

    nc = bacc.Bacc(None, target_bir_lowering=False)
    a3 = nc.declare_dram_parameter("a3", [P, KS, M1], f8, isOutput=False)
    b3 = nc.declare_dram_parameter("b3", [P, KS, N2], f8, isOutput=False)
    out = nc.declare_dram_parameter("out", [M1, N2], f8, isOutput=True)

    with tile.TileContext(nc) as tc:
        with (
            tc.tile_pool(name="persist", bufs=1) as persist,
            tc.tile_pool(name="psa", bufs=2, space=bass.MemorySpace.PSUM) as psa,
            tc.tile_pool(name="psd", bufs=2, space=bass.MemorySpace.PSUM) as psd,
        ):
            a3_t = persist.tile([P, KS, M1], f8, tag="a3t")
            ball = persist.tile([P, KS, N2], f8, tag="ball")
            # h-major loop: one staging tile per output row block, alive
            # for the whole kernel (8 x 8KB/partition)
            ots = [
                persist.tile([P, N2], f8, tag=f"ot{m}", name=f"ot{m}")
                for m in range(MB)
            ]

            # prologue input streaming: pieces in consumption order across
            # all three rings -- small first pieces (512 cols) for early
            # matmul unblock, larger later pieces for ring efficiency
            # (per-DMA fixed cost ~2us dominates small transfers).
            def bp(c0, c1):
                return (ball[:, :, c0:c1], b3[:, :, c0:c1])

            # PE HAM pre-warm setup: memsets lead the gpsimd queue (before
            # its DMA issues) so the dummy matmuls can start at ~8us.
            wdum = persist.tile([P, KS, P], f8, tag="wdum")
            ddum = persist.tile([P, KS, NT], f8, tag="ddum")
            nc.gpsimd.memset(wdum[:], 0.0)
            nc.gpsimd.memset(ddum[:], 0.0)

            # h-major consumption: full a3 + chunk0 (0.5 MB total) is all
            # the input needed to reach the steady rate; later chunks have
            # ~4us of slack each. scalar's queue stays light (a3 only) so
            # its first ACTIVATE isn't pushed out by issue costs.
            nc.scalar.dma_start(a3_t[:, :, 0:P], a3[:, :, 0:P])
            nc.sync.dma_start(*bp(0, 512))
            nc.scalar.dma_start(a3_t[:, :, P:M1], a3[:, :, P:M1])
            nc.sync.dma_start(*bp(512, 1024))
            nc.scalar.dma_start(*bp(1024, 2048))
            nc.sync.dma_start(*bp(2048, 3072))
            nc.scalar.dma_start(*bp(3072, 4096))
            nc.sync.dma_start(*bp(4096, 5120))
            nc.gpsimd.dma_start(*bp(5120, 6144))
            nc.gpsimd.dma_start(*bp(6144, 7168))
            nc.sync.dma_start(*bp(7168, 8192))

            # (no act-table warmup op: Identity is in every table set, so
            # the auto-inserted load attaches to the first real ACTIVATE
            # at the same queue position either way)

            # PE HAM pre-warm: ~2.6us of dummy matmuls on the memset tiles
            # (no input dependency) while the first input pieces stream,
            # so the PE clock is at 8/8 before the real matmuls begin.
            # Cold-clock refills otherwise stall the first ~8 tiles
            # (refill 0.86us+sems > the 1.0us element-op window).
            pdum = psa.tile([P, PW], f32, tag="ps")
            for i in range(6):
                nc.tensor.matmul(
                    pdum[:, (i % 2) * NT : (i % 2 + 1) * NT],
                    wdum[:],
                    ddum[:],
                    start=True,
                    stop=True,
                    perf_mode=DR,
                )

            def tile(h, m):
                ms = slice(m * P, (m + 1) * P)
                c = h * MB + m
                dve = c in DVE_TILES
                ps = (psd if dve else psa).tile([P, PW], f32, tag="ps")
                for j in range(PW // NT):
                    c0 = h * HW + j * NT
                    nc.tensor.matmul(
                        ps[:, j * NT : (j + 1) * NT],
                        a3_t[:, :, ms],
                        ball[:, :, c0 : c0 + NT],
                        start=True,
                        stop=True,
                        perf_mode=DR,
                    )
                oslice = ots[m][:, h * HW : (h + 1) * HW]
                if c == HB * MB - 1:
                    # last tile: split across both engines so the element
                    # streams end together ~0.5us earlier
                    nc.scalar.activation(
                        ots[m][:, 7 * HW : 7 * HW + NT],
                        ps[:, 0:NT],
                        Ident,
                        scale=1.0 / S,
                    )
                    nc.vector.tensor_scalar_mul(
                        ots[m][:, 7 * HW + NT :], ps[:, NT:PW], 1.0 / S
                    )
                elif dve:
                    nc.vector.tensor_scalar_mul(oslice, ps[:], 1.0 / S)
                else:
                    nc.scalar.activation(oslice, ps[:], Ident, scale=1.0 / S)

            # h-major for h<6: output pieces (0.25MB per m, h-pair) stream
            # uniformly on the gpsimd/sync rings through the run
            nout = 0
            for h in range(6):
                for m in range(MB):
                    tile(h, m)
                    if h in (1, 3, 5):
                        ms = slice(m * P, (m + 1) * P)
                        cs = slice((h - 1) * HW, (h + 1) * HW)
                        eng = nc.gpsimd if nout % 2 == 0 else nc.sync
                        nout += 1
                        eng.dma_start(out[ms, cs], ots[m][:, cs])
            # last two h columns per-m as pairs so each row block's final
            # piece leaves immediately; gpsimd stays off the very end (its
            # ~4us SWDGE drain must start early), and the last two blocks'
            # pieces go as halves on sync+scalar concurrently.
            # no SWDGE here: the kernel end is gated by gpsimd's LAST DMA
            # completion + ~4us ring drain + ~4us barrier cascade, so its
            # last piece must come mid-run (h=5), not in the tail. Pair
            # pieces complete ~2.1us apart -- sync alone keeps up.
            for m in range(MB):
                tile(6, m)
                tile(7, m)
                ms = slice(m * P, (m + 1) * P)
                if m < MB - 2:
                    nc.sync.dma_start(out[ms, 6 * HW :], ots[m][:, 6 * HW :])
                elif m == MB - 2:
                    nc.sync.dma_start(
                        out[ms, 6 * HW : 7 * HW], ots[m][:, 6 * HW : 7 * HW]
                    )
                    nc.scalar.dma_start(out[ms, 7 * HW :], ots[m][:, 7 * HW :])
                else:
                    # very last block: 0.125MB quarters, sync||scalar, so
                    # the final transfer's completion is as early as it can
                    H2 = HW // 2
                    for q, eng in enumerate(
                        (nc.sync, nc.scalar, nc.sync, nc.scalar)
                    ):
                        c0 = 6 * HW + q * H2
                        eng.dma_start(
                            out[ms, c0 : c0 + H2], ots[m][:, c0 : c0 + H2]
                        )

    # h-major: weights change every tile, so only the second MM of each
    # tile dedupes (64 survive; the PE's LDW pull-ahead hides them)
    ndrop = _dedupe_ldweights(nc)
    assert ndrop >= 60, f"LDW dedupe removed only {ndrop}"
    nc.finalize()
    return nc


def _prep_inputs(x1, x2):
    """Host-side sharding prep: transpose + fp8 casts; stash sq1/sq2 for
    the decode in _postprocess."""
    global _decode
    import ml_dtypes

    x1 = np.asarray(x1, dtype=np.float32)
    x2 = np.asarray(x2, dtype=np.float32)
    f8 = ml_dtypes.float8_e4m3

    sq1 = (x1.astype(np.float64) ** 2).sum(axis=1).astype(np.float32)
    sq2 = (x2.astype(np.float64) ** 2).sum(axis=1).astype(np.float32)
    _decode = (sq1, sq2)

    # [p, s, n] layout: k = s*128 + p
    a3_all = np.ascontiguousarray(
        (-2.0 * x1).T.reshape(KS, P, N1).transpose(1, 0, 2).astype(f8)
    )  # [P, KS, N1]
    b3 = np.ascontiguousarray(
        x2.T.reshape(KS, P, N2).transpose(1, 0, 2).astype(f8)
    )  # [P, KS, N2]

    in_maps = []
    for c in range(NCORES):
        sl = slice(c * M1, (c + 1) * M1)
        in_maps.append(
            {
                "a3": np.ascontiguousarray(a3_all[:, :, sl]),
                "b3": b3,
            }
        )
    return in_maps


def _postprocess(res):
    """Unshard + decode: d = sqrt(S*v + sq1_i + sq2_j)."""
    sq1, sq2 = _decode
    v = np.concatenate(
        [np.asarray(res.results[c]["out"]) for c in range(NCORES)], axis=0
    ).astype(np.float32)
    v *= S
    v += sq1[:, None]
    v += sq2[None, :]
    np.maximum(v, 0.0, out=v)
    return np.sqrt(v, out=v)


def _run(in_maps, trace=False):
    global _built
    from concourse.bass_utils import run_bass_kernel_spmd

    if _built is None:
        _built = _build_nc()
    return run_bass_kernel_spmd(_built, in_maps, list(range(NCORES)), trace=trace)


def kernel(x1, x2):
    in_maps = _prep_inputs(x1, x2)
    res = _run(in_maps, trace=False)
    return _postprocess(res)


# revision 49
# speedup vs baseline: 1.0443x; 1.0443x over previous
"""Euclidean distance matrix (torch.cdist p=2) on 8 Trainium2 NeuronCores.

Strategy — fp8 residual output, no on-device sqrt (~55us vs 80us for the
fp16+device-sqrt baseline):
  - d^2 = ||a||^2 + ||b||^2 - 2 a.b. Only the cross term needs the device;
    sq1 (per row) and sq2 (per column) are tiny host-side vectors added
    exactly during decode. The device outputs v = (-2 a.b)/S in fp8e4m3
    and the host computes d = sqrt(S*v + sq1_i + sq2_j). cross is
    zero-centered, so fp8 quantization of v adds ~7e-3 rel err on top of
    the ~6e-3 from fp8 matmul inputs (measured 1.35e-2 total vs the 2e-2
    gate; S=48 keeps |v| <= ~4.2, in the fine fp8 octaves).
  - Why: the baseline was walled by Scalar-engine Sqrt (60us of ACTIVATE,
    the only sqrt engine) and by HBM traffic (16.8 MB fp16 out + 2.3 MB
    in ~ 53us floor at ~358 GB/s/core). Dropping sqrt lets BOTH
    PSUM-capable element engines split the PSUM->SBUF drain: ACT runs
    Identity activation with scale=1/S (1.00us per [128,1024] tile,
    (172+FD)/1.2ns) and DVE runs tensor_scalar_mul (1.13us,
    (120+FD)/0.96ns) -- 34.5us each in parallel, the kernel's wall.
    fp8 output halves DMA bytes (8.4 MB out).
  - PE: cross matmuls only (fp8e4m3 DoubleRow, K=2x128, N=512, 2 per
    psum tile), ~35us busy -- slack vs the element wall, which also
    absorbs its HAM clock oscillation (short idle gaps keep it at
    K=4/8 half-clock part of the time).
  - PSUM: each element engine owns a private 2-buf pool of [128,1024]
    (2 banks x 4 = all 8 banks). A shared round-robin pool stalls PE on
    the specific buffer held by the slower engine (~+0.5us/pair); with
    private pools each engine's next buffer refills during its current
    op, and both streams run back-to-back (ACT 95%+, DVE 98%+ occupancy).
  - Loop is h-major (64 tiles, c = h*MB + m): full-rate compute needs
    only a3 (0.25 MB) + the first column chunk, so the input ramp is
    ~6us instead of ~12 (m-major needs all 2.25 MB before m=0 ends).
    Weights then change every tile; the PE's LDWEIGHTS pull-ahead hides
    the reloads (~0.14us effective). Tiles alternate ACT/DVE by parity
    (flips at {9,33} give the 34/30 split matching the 1.00:1.13 rates;
    keeping c=1 on DVE lets both pipelines start immediately).
  - Output: per-m staging tiles [128, 8192] fp8 live all run; 0.25 MB
    pieces per (m, h-pair) leave on gpsimd/sync through the run. h=6,7
    run per-m as pairs with their piece right behind, the last two split
    sync||scalar. gpsimd (SWDGE) carries nothing after h=5: the kernel
    end is gated by its last DMA + ~4us ring drain + ~4us barrier
    cascade. Inputs stream on sync+scalar HWDGE in consumption order
    (512-col first pieces); early SWDGE input traffic would steal SDMA
    round-robin bandwidth from the latency-critical first pieces.
"""

import numpy as np

N1 = 8192  # x1 rows (output rows)
N2 = 8192  # x2 rows (output cols)
D = 256    # feature dim
NCORES = 8
M1 = N1 // NCORES  # 1024 output rows per core
P = 128            # partitions
KS = 2             # fp8 DoubleRow k-subtiles (K = KS*P = 256)
NT = 512           # matmul moving free dim (one PSUM bank)
PW = 1024          # psum tile width (2 banks); 4 bufs = full PSUM
MB = M1 // P       # 8 output-row blocks per core
HB = 8             # column tiles per row block (1024 cols each)
HW = N2 // HB      # 1024
S = 48.0           # fp8 output scale: v = psum/S, |v| <= ~4.2

# per-tile element-engine assignment (c = h*MB + m, 64 tiles):
# DVE on odd tiles except {9,33} -> 34 ACT / 30 DVE, matching the
# measured 1.00 (ACT) : 1.13 (DVE) us/tile rates (tile 63 is split
# across both engines so the streams end together).
DVE_TILES = frozenset(c for c in range(64) if c % 2 == 1 and c not in (9, 33))

_built = None
_decode = None  # (sq1, sq2) stashed by _prep_inputs for _postprocess


def _ldw_key(inst):
    return (
        str(inst.ins[0]),
        str(getattr(inst, "perf_mode", None)),
        str(getattr(inst, "tile_position", None)),
    )


def _dedupe_ldweights(nc):
    """Drop InstLdweights whose weights AP equals the currently-loaded one
    (no different load in between on the PE stream). Their rare sync waits
    are migrated to the next PE instruction; Bacc.finalize() later splits
    any resulting multi-wait into EventSemaphore preludes."""
    import concourse.mybir as mybir

    dropped = 0
    for f in nc.m.functions:
        for blk in f.blocks:
            insts = list(blk.instructions)
            cur_key = None
            pending = []
            to_drop = []
            for inst in insts:
                if isinstance(inst, mybir.InstLdweights):
                    key = _ldw_key(inst)
                    if key == cur_key:
                        si = inst.sync_info
                        if si is not None and si.on_wait:
                            pending.extend(si.on_wait)
                        to_drop.append(inst)
                    else:
                        cur_key = key
                elif isinstance(inst, mybir.InstMatmult):
                    if pending:
                        si = inst.sync_info
                        waits = list(si.on_wait) if si else []
                        upds = list(si.on_update) if si else []
                        inst.sync_info = mybir.SyncInfo(
                            on_wait=waits + pending, on_update=upds
                        )
                        pending = []
            assert not pending
            for inst in to_drop:
                blk.instructions.remove(inst)
            dropped += len(to_drop)
    return dropped


def _build_nc():
    import concourse.bass as bass
    import concourse.mybir as mybir
    from concourse import bacc, tile

    f8 = mybir.dt.float8e4
    f32 = mybir.dt.float32
    DR = mybir.MatmulPerfMode.DoubleRow
    Ident = mybir.ActivationFunctionType.Identity

    nc = bacc.Bacc(None, target_bir_lowering=False)
    a3 = nc.declare_dram_parameter("a3", [P, KS, M1], f8, isOutput=False)
    b3 = nc.declare_dram_parameter("b3", [P, KS, N2], f8, isOutput=False)
    out = nc.declare_dram_parameter("out", [M1, N2], f8, isOutput=True)

    with tile.TileContext(nc) as tc:
        with (
            tc.tile_pool(name="persist", bufs=1) as persist,
            tc.tile_pool(name="psa", bufs=2, space=bass.MemorySpace.PSUM) as psa,
            tc.tile_pool(name="psd", bufs=2, space=bass.MemorySpace.PSUM) as psd,
        ):
            a3_t = persist.tile([P, KS, M1], f8, tag="a3t")
            ball = persist.tile([P, KS, N2], f8, tag="ball")
            # h-major loop: one staging tile per output row block, alive
            # for the whole kernel (8 x 8KB/partition)
            ots = [
                persist.tile([P, N2], f8, tag=f"ot{m}", name=f"ot{m}")
                for m in range(MB)
            ]

            # prologue input streaming: pieces in consumption order across
            # all three rings -- small first pieces (512 cols) for early
            # matmul unblock, larger later pieces for ring efficiency
            # (per-DMA fixed cost ~2us dominates small transfers).
            def bp(c0, c1):
                return (ball[:, :, c0:c1], b3[:, :, c0:c1])

            # PE HAM pre-warm setup: memsets lead the gpsimd queue (before
            # its DMA issues) so the dummy matmuls can start at ~8us.
            wdum = persist.tile([P, KS, P], f8, tag="wdum")
            ddum = persist.tile([P, KS, NT], f8, tag="ddum")
            nc.gpsimd.memset(wdum[:], 0.0)
            nc.gpsimd.memset(ddum[:], 0.0)

            # h-major consumption: full a3 + chunk0 (0.5 MB total) is all
            # the input needed to reach the steady rate; later chunks have
            # ~4us of slack each. scalar's queue stays light (a3 only) so
            # its first ACTIVATE isn't pushed out by issue costs.
            nc.scalar.dma_start(a3_t[:, :, 0:P], a3[:, :, 0:P])
            nc.sync.dma_start(*bp(0, 512))
            nc.scalar.dma_start(a3_t[:, :, P:M1], a3[:, :, P:M1])
            nc.sync.dma_start(*bp(512, 1024))
            nc.scalar.dma_start(*bp(1024, 2048))
            nc.sync.dma_start(*bp(2048, 3072))
            nc.scalar.dma_start(*bp(3072, 4096))
            nc.sync.dma_start(*bp(4096, 5120))
            nc.gpsimd.dma_start(*bp(5120, 6144))
            nc.gpsimd.dma_start(*bp(6144, 7168))
            nc.sync.dma_start(*bp(7168, 8192))

            # (no act-table warmup op: Identity is in every table set, so
            # the auto-inserted load attaches to the first real ACTIVATE
            # at the same queue position either way)

            # PE HAM pre-warm: ~2.6us of dummy matmuls on the memset tiles
            # (no input dependency) while the first input pieces stream,
            # so the PE clock is at 8/8 before the real matmuls begin.
            # Cold-clock refills otherwise stall the first ~8 tiles
            # (refill 0.86us+sems > the 1.0us element-op window).
            pdum = psa.tile([P, PW], f32, tag="ps")
            for i in range(6):
                nc.tensor.matmul(
                    pdum[:, (i % 2) * NT : (i % 2 + 1) * NT],
                    wdum[:],
                    ddum[:],
                    start=True,
                    stop=True,
                    perf_mode=DR,
                )

            def tile(h, m):
                ms = slice(m * P, (m + 1) * P)
                c = h * MB + m
                dve = c in DVE_TILES
                ps = (psd if dve else psa).tile([P, PW], f32, tag="ps")
                for j in range(PW // NT):
                    c0 = h * HW + j * NT
                    nc.tensor.matmul(
                        ps[:, j * NT : (j + 1) * NT],
                        a3_t[:, :, ms],
                        ball[:, :, c0 : c0 + NT],
                        start=True,
                        stop=True,
                        perf_mode=DR,
                    )
                oslice = ots[m][:, h * HW : (h + 1) * HW]
                if c == HB * MB - 1:
                    # last tile: split across both engines so the element
                    # streams end together ~0.5us earlier
                    nc.scalar.activation(
                        ots[m][:, 7 * HW : 7 * HW + NT],
                        ps[:, 0:NT],
                        Ident,
                        scale=1.0 / S,
                    )
                    nc.vector.tensor_scalar_mul(
                        ots[m][:, 7 * HW + NT :], ps[:, NT:PW], 1.0 / S
                    )
                elif dve:
                    nc.vector.tensor_scalar_mul(oslice, ps[:], 1.0 / S)
                else:
                    nc.scalar.activation(oslice, ps[:], Ident, scale=1.0 / S)

            # h-major for h<6: output pieces (0.25MB per m, h-pair) stream
            # uniformly on the gpsimd/sync rings through the run
            nout = 0
            for h in range(6):
                for m in range(MB):
                    tile(h, m)
                    if h in (1, 3, 5):
                        ms = slice(m * P, (m + 1) * P)
                        cs = slice((h - 1) * HW, (h + 1) * HW)
                        eng = nc.gpsimd if nout % 2 == 0 else nc.sync
                        nout += 1
                        eng.dma_start(out[ms, cs], ots[m][:, cs])
            # last two h columns per-m as pairs so each row block's final
            # piece leaves immediately; gpsimd stays off the very end (its
            # ~4us SWDGE drain must start early), and the last two blocks'
            # pieces go as halves on sync+scalar concurrently.
            # no SWDGE here: the kernel end is gated by gpsimd's LAST DMA
            # completion + ~4us ring drain + ~4us barrier cascade, so its
            # last piece must come mid-run (h=5), not in the tail. Pair
            # pieces complete ~2.1us apart -- sync alone keeps up.
            for m in range(MB):
                tile(6, m)
                tile(7, m)
                ms = slice(m * P, (m + 1) * P)
                if m < MB - 2:
                    nc.sync.dma_start(out[ms, 6 * HW :], ots[m][:, 6 * HW :])
                elif m == MB - 2:
                    nc.sync.dma_start(
                        out[ms, 6 * HW : 7 * HW], ots[m][:, 6 * HW : 7 * HW]
                    )
                    nc.scalar.dma_start(out[ms, 7 * HW :], ots[m][:, 7 * HW :])
                else:
                    # very last block: 0.125MB quarters, sync||scalar, so
                    # the final transfer's completion is as early as it can
                    H2 = HW // 2
                    for q, eng in enumerate(
                        (nc.sync, nc.scalar, nc.sync, nc.scalar)
                    ):
                        c0 = 6 * HW + q * H2
                        eng.dma_start(
                            out[ms, c0 : c0 + H2], ots[m][:, c0 : c0 + H2]
                        )

    # h-major: weights change every tile, so only the second MM of each
    # tile dedupes (64 survive; the PE's LDW pull-ahead hides them)
    ndrop = _dedupe_ldweights(nc)
    assert ndrop >= 60, f"LDW dedupe removed only {ndrop}"
    nc.finalize()
    return nc


def _prep_inputs(x1, x2):
    """Host-side sharding prep: transpose + fp8 casts; stash sq1/sq2 for
    the decode in _postprocess."""
    global _decode
    import ml_dtypes

    x1 = np.asarray(x1, dtype=np.float32)
    x2 = np.asarray(x2, dtype=np.float32)
    f8 = ml_dtypes.float8_e4m3

    sq1 = (x1.astype(np.float64) ** 2).sum(axis=1).astype(np.float32)
    sq2 = (x2.astype(np.float64) ** 2).sum(axis=1).astype(np.float32)
    _decode = (sq1, sq2)

    # [p, s, n] layout: k = s*128 + p
    a3_all = np.ascontiguousarray(
        (-2.0 * x1).T.reshape(KS, P, N1).transpose(1, 0, 2).astype(f8)
    )  # [P, KS, N1]
    b3 = np.ascontiguousarray(
        x2.T.reshape(KS, P, N2).transpose(1, 0, 2).astype(f8)
    )  # [P, KS, N2]

    in_maps = []
    for c in range(NCORES):
        sl = slice(c * M1, (c + 1) * M1)
        in_maps.append(
            {
                "a3": np.ascontiguousarray(a3_all[:, :, sl]),
                "b3": b3,
            }
        )
    return in_maps


def _postprocess(res):
    """Unshard + decode: d = sqrt(S*v + sq1_i + sq2_j)."""
    sq1, sq2 = _decode
    v = np.concatenate(
        [np.asarray(res.results[c]["out"]) for c in range(NCORES)], axis=0
    ).astype(np.float32)
    v *= S
    v += sq1[:, None]
    v += sq2[None, :]
    np.maximum(v, 0.0, out=v)
    return np.sqrt(v, out=v)


def _run(in_maps, trace=False):
    global _built
    from concourse.bass_utils import run_bass_kernel_spmd

    if _built is None:
        _built = _build_nc()
    return run_bass_kernel_spmd(_built, in_maps, list(range(NCORES)), trace=trace)


def kernel(x1, x2):
    in_maps = _prep_inputs(x1, x2)
    res = _run(in_maps, trace=False)
    return _postprocess(res)
